# revision 39
# baseline (speedup 1.0000x reference)
"""Trainium2 Bass kernel for causal multi-head attention (GPT-style block).

Reference computation (fp32):
    qkv = x @ w_attn + b_attn          # [B,T,3C]
    q,k,v per head (12 heads, d=64)
    att = softmax(causal(q k^T / 8))
    y   = att @ v
    out = y @ w_proj + b_proj

Sharding: 8 cores = 2 batches x 4 head-groups (3 heads each).
Tensor-parallel over heads: each core takes its 3 heads' columns of
w_attn and rows of w_proj, computes a partial out^T [C, T]; the host
sums the 4 head-group partials per batch, transposes, adds b_proj.

Device kernel (per core), all matmuls bf16, PSUM fp32:

  1. Inputs land via few, large DMAs spread over 4 sequencer rings
     (sync/vector/scalar/gpsimd) so dispatch (~0.65us per dma_start)
     isn't serial.  x^T arrives as 6 per-c [128, T] tiles, split
     [0:512] (hot: feeds chunk-0 qkv) + [512:2048].
  2. While inputs stream, the PE runs warm-up matmuls on a zeroed
     scratch tile: the Tensor engine needs ~3us of continuous busy to
     ramp from 1.2GHz to 2.4GHz, so the ramp happens during the DMA
     window instead of during the first real chains.
  3. Q/K^T = w^T x^T in 3 M-blocks: b0=[Q0|Q1] b1=[K0|K1] b2=[Q2|K2]
     ([128,T] SBUF tiles, | = partition 64); bias add on DVE. K2 is
     shifted to partition base 0 via an SBUF-SBUF DMA (matmul requires
     lhsT/rhs partition bases to match).
  4. V computed TOKEN-major directly: V[t,d] = x^T-tile.T @ wv
     -> vk k-tiles [128, 3*65] (64 V cols + ones col per head; the
     ones column turns the AV matmul into a fused softmax-denominator
     sum).
  5. attention rounds r=(qc,j), software-pipelined two rounds deep:
       S^T = K-slice.T @ Q   (PE, causal-sliced N, 4-slot PSUM rotation)
       eS = exp(0.125 S^T)    (ACT - the pacing engine of attention)
       diag strip of eS zeroed by a 0/1 triangle multiply (DVE) AFTER
       the exp, so the exp chain never waits on the mask
       AV of round r-2: y^T[65,q] += [V|1].T @ eS  (PE)
     qkv/V chains are dispatched between rounds from a deadline queue
     as PE gap fillers.  proj chains are RESERVED for the last
     q-chunk's rounds: that phase has no qkv/V fills left and is
     otherwise ACT-(exp-)bound, so proj keeps the PE dense there.
  6. drain per q-chunk (boundary-latency critical): ONE Pool copy
     moves l + unnormalized y^T out of PSUM so the next chunk's AV can
     reuse the bank immediately; 1/l runs off the critical path
     (mid-kernel: DMA-shift l row to partition 0 + DVE reciprocal +
     GpSimd partition-broadcast; tail: PE K=1 broadcast for minimum
     latency), then y^T *= 1/l on Pool.  (Custom-DVE reciprocal and
     gpsimd partition_broadcast are broken on HW at partition base
     != 0 - everything runs at base 0.)
  7. proj emits out^T: pso[128n,512t] = wp-slice.T @ y^T  (PE, N=512),
     engine copy to SBUF, DMA out (rings rotated).  A few proj units
     are held back to keep the PE busy under the tail drain's
     reciprocal latency.  Host transposes + reduces partials.
"""

import sys

import numpy as np

sys.path.insert(0, "/opt/trn_rl_repo")

from concourse import bacc, bass, mybir  # noqa: E402
from concourse import bass_utils  # noqa: E402
from concourse.tile import TileContext  # noqa: E402

P = 128
T = 2048
CDIM = 768
DHEAD = 64
HPC = 3  # heads per core
N_CORES = 8
FQK = 384  # q+k features per core (3 blocks of 128)
FV = 192  # v features per core
QW = 512  # q-chunk width
NQC = T // QW  # 4
NTT = T // P  # 16 token tiles
NCT = CDIM // P  # 6 contraction tiles
SCALE = 1.0 / np.sqrt(DHEAD)

N_WARM = 13  # PE clock warm-up matmuls; end ~= hot-x arrival so the busy
# streak is unbroken and the qkv chains start at max clock
PROJ_TAIL_RESERVE = 6  # proj units held for the tail drain window

DT = mybir.dt.float32
DTM = mybir.dt.bfloat16


def build_nc():
    from contextlib import ExitStack

    nc = bacc.Bacc("TRN2", target_bir_lowering=False, debug=False)
    x_d = nc.dram_tensor("x", [CDIM, T], DTM, kind="ExternalInput")
    w_d = nc.dram_tensor("w", [CDIM, FQK + FV], DTM, kind="ExternalInput")
    bqk_d = nc.dram_tensor("bqk", [3, P], DT, kind="ExternalInput")
    bvb_d = nc.dram_tensor("bvb", [P, FV], DT, kind="ExternalInput")
    wp_d = nc.dram_tensor("wp", [HPC * DHEAD, CDIM], DTM, kind="ExternalInput")
    m_d = nc.dram_tensor("mask", [P, P], DTM, kind="ExternalInput")
    onr_d = nc.dram_tensor("ones_r", [1, 64], mybir.dt.float32r, kind="ExternalInput")
    o_d = nc.dram_tensor("out", [CDIM, T], DT, kind="ExternalOutput")

    EXP = mybir.ActivationFunctionType.Exp

    with TileContext(nc) as tc, ExitStack() as ctx:
        const = ctx.enter_context(tc.tile_pool(name="const", bufs=1))
        big = ctx.enter_context(tc.tile_pool(name="big", bufs=1))
        work = ctx.enter_context(tc.tile_pool(name="work", bufs=4))
        psyp = ctx.enter_context(
            tc.tile_pool(name="psy", bufs=1, space=bass.MemorySpace.PSUM)
        )
        pssp = ctx.enter_context(
            tc.tile_pool(name="pss", bufs=1, space=bass.MemorySpace.PSUM)
        )
        misc = ctx.enter_context(
            tc.tile_pool(name="misc", bufs=1, space=bass.MemorySpace.PSUM)
        )

        # ---- input DMAs: few + large, spread over 4 rings ----
        # x^T: 6 per-c tiles [128, T]; cols [0:512] first (hot), rest after
        xT = [big.tile([P, T], DTM, tag=f"xT{c}", name=f"xT{c}") for c in range(NCT)]

        def x_ap(c, lo, hi):
            return xT[c][:, lo:hi]

        # warm-up scratch FIRST on the gpsimd ring so the PE can start
        # ramping at t~6.2us with no DMA dependency
        scratch = const.tile([P, QW], DTM, tag="scratch")
        nc.gpsimd.memset(scratch[:], 0.0)

        # Many medium DMAs spread over the 3 DMA-capable rings: each
        # dma_start lands on its own hw queue, so parallelism comes from
        # instruction count, not size.  Hot pieces (w, x t0) dispatch first.
        FW = FQK + FV
        wbig = const.tile([P, NCT * FW], DTM, tag="wbig")
        for c in range(NCT):
            # w c-blocks on scalar (in chain-visit order)
            nc.scalar.dma_start(
                wbig[:, c * FW : (c + 1) * FW], w_d[c * P : (c + 1) * P, :]
            )
        # hot x [0:512]: the minimal set for qkv(0); sync c0-3, gpsimd c4-5
        for c in range(4):
            nc.sync.dma_start(xT[c][:, 0:QW], x_d[c * P : (c + 1) * P, 0:QW])
        for c in range(4, NCT):
            nc.gpsimd.dma_start(xT[c][:, 0:QW], x_d[c * P : (c + 1) * P, 0:QW])
        # small consts on gpsimd (mask gates the first exp round)
        bqk_t = const.tile([P, 3], DT, tag="bqk")
        nc.gpsimd.dma_start(bqk_t[:], bqk_d[:].rearrange("b p -> p b"))
        mask_t = const.tile([P, P], DTM, tag="mask")
        nc.gpsimd.dma_start(mask_t[:], m_d[:])
        bvb_t = const.tile([P, FV], DT, tag="bvb")
        nc.gpsimd.dma_start(bvb_t[:], bvb_d[:])

        # ---- 4-bank S-PSUM strip, manually slot-rotated: slot (3r+h)%4.
        # Rounds whose 3 slots are contiguous get ONE fused exp over all
        # heads (wrapping rounds get 2) - each fused exp saves ~190ns of
        # ACT overhead, and ACT paces the attention phases.
        pb = pssp.tile([P, 4 * QW], DT, tag="pss", name="pb")

        # ---- PE warm-up: ramp the Tensor clock during the DMA window ----
        for _ in range(N_WARM):
            nc.tensor.matmul(
                pb[:, 0:QW], scratch[:, 0:P], scratch[:], start=True, stop=True
            )

        # ---- V k-tiles; ones column per head fused for the l-sum ----
        vk = [
            big.tile([P, HPC * 65], DTM, tag=f"vk{j}", name=f"vk{j}")
            for j in range(NTT)
        ]
        for j in range(NTT):
            ones_view = vk[j][:].rearrange("p (h c) -> p h c", c=65)[:, :, 64:65]
            nc.gpsimd.memset(ones_view, 1.0)

        ones_r = const.tile([65, 64], mybir.dt.float32r, tag="ones_r")
        nc.gpsimd.dma_start(ones_r[64:65, :], onr_d[:])

        # ---- chain emitters (each is a PE filler unit) ----
        blk = [big.tile([P, T], DTM, tag=f"blk{i}", name=f"blk{i}") for i in range(3)]
        # K2 copy at partition base 0 (matmul needs lhsT/rhs bases equal;
        # Q2 lives at base 0 in blk2, K2 at base 64 -> shift via DMA)
        kT2 = big.tile([64, T], DTM, tag="kT2", name="kT2")

        qkv_state = {}

        def qkv_unit(t, bi, half):
            tcols = slice(t * QW, (t + 1) * QW)
            if half == 0:
                qkv_state[(t, bi)] = misc.tile([P, QW], DT, tag="misc", name="psq")
            psq = qkv_state[(t, bi)]
            for c in (half * 2, half * 2 + 1) if half < 2 else (4, 5):
                nc.tensor.matmul(
                    psq[:],
                    wbig[:, c * FW + bi * P : c * FW + (bi + 1) * P],
                    x_ap(c, t * QW, (t + 1) * QW),
                    start=(c == 0),
                    stop=(c == NCT - 1),
                )
            if half == 2:
                nc.vector.tensor_scalar_add(
                    blk[bi][:, tcols], psq[:], bqk_t[:, bi : bi + 1]
                )
                if bi == 2:
                    nc.sync.dma_start(kT2[:, tcols], blk[2][64:P, tcols])
            return 540

        v_state = {}

        def v_unit(j, half):
            if half == 0:
                v_state[j] = misc.tile([P, FV], DT, tag="misc", name="pst")
            pst = v_state[j]
            for c in (half * 2, half * 2 + 1) if half < 2 else (4, 5):
                nc.tensor.matmul(
                    pst[:],
                    x_ap(c, j * P, (j + 1) * P),
                    wbig[:, c * FW + FQK : c * FW + FQK + FV],
                    start=(c == 0),
                    stop=(c == NCT - 1),
                )
            if half == 2:
                data_view = vk[j][:].rearrange("p (h c) -> p h c", c=65)[:, :, 0:64]
                nc.vector.tensor_add(
                    data_view,
                    pst[:].rearrange("p (h c) -> p h c", c=64),
                    bvb_t[:].rearrange("p (h c) -> p h c", c=64),
                )
            return 270

        # ---- attention machinery ----
        qsrc = [(0, 0), (0, 64), (2, 0)]
        ktile = [blk[1], blk[1], kT2]
        krow = [0, 64, 0]
        yT0 = big.tile([P, T], DTM, tag="yT0")  # rows: h0 | h1
        yT1 = big.tile([64, T], DTM, tag="yT1")  # h2
        psy = [None, None, None]

        ri_ctr = [0]

        def emit_s_round(qc, j):
            m = j - 4 * qc
            cs = m * P if m >= 1 else 0
            ssl = slice(cs, QW)
            qsl = slice(qc * QW + cs, (qc + 1) * QW)
            ri = ri_ctr[0]
            ri_ctr[0] += 1
            slots = [(3 * ri + h) % 4 for h in range(HPC)]
            for h in range(HPC):
                qb, qr = qsrc[h]
                kr = krow[h]
                s = slots[h]
                nc.tensor.matmul(
                    pb[:, s * QW + cs : (s + 1) * QW],
                    ktile[h][kr : kr + 64, j * P : (j + 1) * P],
                    blk[qb][qr : qr + 64, qsl],
                    start=True,
                    stop=True,
                )
            es = work.tile([P, HPC * QW], DTM, tag="es", bufs=5, name="es")
            pbv = pb[:].rearrange("p (s q) -> p s q", q=QW)
            esv = es[:].rearrange("p (h q) -> p h q", q=QW)
            # fused exps over contiguous slot runs
            h = 0
            while h < HPC:
                n = 1
                while h + n < HPC and slots[h + n] == slots[h] + n:
                    n += 1
                nc.scalar.activation(
                    esv[:, h : h + n, ssl],
                    pbv[:, slots[h] : slots[h] + n, ssl],
                    EXP,
                    scale=float(SCALE),
                )
                h += n
            if m >= 0:
                # causal mask applied AFTER exp (multiply by 0/1 triangle)
                # so the exp never waits on it
                msl = slice(m * P, (m + 1) * P)
                for h in range(HPC):
                    nc.vector.tensor_mul(
                        esv[:, h, msl], esv[:, h, msl], mask_t[:]
                    )
            return (qc, j, es, ssl)

        def emit_av_round(qc, j, es, ssl):
            first = j == 0
            last = j == 4 * (qc + 1) - 1
            if first:
                for h in range(HPC):
                    psy[h] = psyp.tile([65, QW], DT, tag=f"psy{h}", name=f"psy{h}")
            for h in range(HPC):
                nc.tensor.matmul(
                    psy[h][:, ssl],
                    vk[j][:, 65 * h : 65 * h + 65],
                    es[:, h * QW + ssl.start : (h + 1) * QW],
                    start=first,
                    stop=last,
                )
            return last

        def drain_unit(qc, h):
            # Free psy FAST: one DVE copy moves l + unnormalized y out of
            # PSUM so the next q-chunk's AV can reuse the bank immediately.
            # The 1/l chain (reciprocal at base 0 + broadcast) runs off the
            # critical path; only proj(qc) waits on it.
            qcols = slice(qc * QW, (qc + 1) * QW)
            if h == 0:
                ydst = yT0[0:64, qcols]
            elif h == 2:
                ydst = yT1[0:64, qcols]
            else:
                tmp = work.tile([64, QW], DTM, tag="ytmp", bufs=3)
                ydst = tmp[:]
            rb = work.tile([64, QW], DT, tag="rb", bufs=4)
            if qc < NQC - 1:
                # mid-kernel: keep the broadcast off the PE (the busy
                # engine); DMA-shift l to partition 0, reciprocal there,
                # gpsimd partition-broadcast. Latency is hidden: only
                # proj(qc) waits on rb.
                st = work.tile([65, QW], mybir.dt.float32r, tag="lr", bufs=4)
                nc.vector.tensor_copy(st[:], psy[h][:])
                lr0 = work.tile([1, QW], DT, tag="lr0", bufs=2)
                nc.sync.dma_start(lr0[:], st[64:65, :].bitcast(DT))
                rc = work.tile([1, QW], DT, tag="rc", bufs=2)
                nc.vector.reciprocal_approx_fast(out=rc[:], in_=lr0[:])
                nc.gpsimd.partition_broadcast(rb[:], rc[0:1, :], channels=64)
                nc.vector.tensor_mul(ydst, st[0:64, :].bitcast(DT), rb[:])
            else:
                # tail: latency-critical, psy needs no handoff.  Skip the
                # full copy: ACT (idle now) moves just the l row to a
                # base-64 SBUF slot, PE K=1 broadcast, reciprocal, and
                # normalize reading y straight from PSUM on the DVE.
                lr64 = tail_lr[h]
                s = {1: 0, 0: 1, 2: 2}[h]
                rbp = pb[0:64, s * QW : s * QW + QW]
                nc.tensor.matmul(
                    rbp, ones_r[64:65, :], lr64[64:65, :], start=True, stop=True
                )
                nc.vector.reciprocal_approx_fast(out=rb[:], in_=rbp)
                nc.vector.tensor_mul(ydst, psy[h][0:64, :], rb[:])
            if h == 1:
                nc.sync.dma_start(yT0[64:P, qcols], tmp[:])
            return 300

        tail_lr = {}

        def drain_lcopy(h):
            # ACT (idle at the tail) moves just the l row to a base-64 slot
            lr64 = work.tile([65, QW], mybir.dt.float32r, tag="lr64", bufs=3)
            nc.scalar.copy(lr64[64:65, :], psy[h][64:65, :])
            tail_lr[h] = lr64

        proj_n = [0]  # running proj-unit counter for ring/engine rotation

        def proj_unit(qc, ns, tail=False):
            tsl = slice(qc * QW, (qc + 1) * QW)
            nsl = slice(ns * P, (ns + 1) * P)
            if tail:
                s = (3 + proj_n[0]) % 4
                pso = pb[:, s * QW : s * QW + QW]
            else:
                pso = misc.tile([P, QW], DT, tag="misc", name="pso")[:]
            nc.tensor.matmul(pso, wp0[:, nsl], yT0[:, tsl], start=True, stop=False)
            nc.tensor.matmul(pso, wp1[:, nsl], yT1[:, tsl], start=False, stop=True)
            ot = work.tile([P, QW], DT, tag="ot", bufs=4)
            k = proj_n[0]
            proj_n[0] += 1
            if tail:
                # tail: exps done -> ACT is idle; copies mostly on ACT (the
                # DVE carries the drain recip/normalize chain), DMAs
                # alternate sync/scalar
                eng = (nc.scalar.copy, nc.scalar.copy, nc.vector.tensor_copy)[k % 3]
                eng(ot[:], pso)
                ring = (nc.sync, nc.scalar)[k % 2]
            else:
                # mid-kernel: ACT is the pacer - keep copies off it (DVE;
                # Pool cannot read PSUM)
                nc.vector.tensor_copy(ot[:], pso)
                ring = (nc.sync, nc.gpsimd)[k % 2]
            ring.dma_start(o_d[nsl, tsl], ot[:])
            return 520

        # ---- prologue: only what round (0,0) needs ----
        for bi in range(3):
            for half in range(3):
                qkv_unit(0, bi, half)

        # mid x [512:1024] (qkv(1), rounds 2-3) then cold [1024:2048]
        # (2KB descriptors), spread sync/scalar/gpsimd behind the hot set
        H2 = 2 * QW
        rings = [nc.sync, nc.sync, nc.scalar, nc.scalar, nc.gpsimd, nc.gpsimd]
        for c in range(NCT):
            rings[c].dma_start(
                xT[c][:, QW:H2], x_d[c * P : (c + 1) * P, QW:H2]
            )
        for c in range(NCT):
            rings[c].dma_start(
                xT[c][:, H2:T], x_d[c * P : (c + 1) * P, H2:T]
            )
        wp0 = const.tile([P, CDIM], DTM, tag="wp0")
        nc.gpsimd.dma_start(wp0[:], wp_d[0:P, :])
        wp1 = const.tile([64, CDIM], DTM, tag="wp1")
        nc.gpsimd.dma_start(wp1[:], wp_d[P : P + 64, :])

        def qkv_chain(t, bi):
            for half in range(3):
                qkv_unit(t, bi, half)
            return 1620

        def v_chain(j):
            for half in range(3):
                v_unit(j, half)
            return 810

        # ---- filler queue with deadlines (baseline scheme) ----
        # each entry: (deadline_round_index, thunk). Chains pop atomically;
        # pacing is even distribution (PE total work > ACT total, so the
        # queue must drain by the last round, not just plug ACT gaps).
        # The last drained chunk holds PROJ_TAIL_RESERVE proj units back
        # for the tail, to keep the PE busy under the drain's recip latency.
        rounds = [(qc, j) for qc in range(NQC) for j in range(4 * (qc + 1))]
        ridx = {r: i for i, r in enumerate(rounds)}
        NR = len(rounds)
        queue = []
        for j in range(4):
            queue.append((ridx[(0, j)], lambda j=j: v_chain(j)))
        for t in range(1, 4):
            for bi in range(3):
                queue.append((ridx[(t, 0)] - 2, lambda t=t, bi=bi: qkv_chain(t, bi)))
        for j in range(4, NTT):
            queue.append((ridx[(j // 4, j)], lambda j=j: v_chain(j)))
        queue.sort(key=lambda e: e[0])
        reserve = []

        # ---- main loop: S(r) + AV(r-2) + evenly-paced fillers ----
        pends = []
        for ri, (qc, j) in enumerate(rounds):
            cur = emit_s_round(qc, j)
            # at a qc boundary flush BOTH pending AV rounds so the drain
            # (and the psy bank handoff) starts a full round earlier
            flush = 2 if (j == 0 and pends) else 1
            for _ in range(flush):
                if len(pends) < (3 - flush):
                    break
                if not pends:
                    break
                pend = pends.pop(0)
                was_last = emit_av_round(*pend)
                if was_last:
                    pqc = pend[0]
                    for h in (1, 0, 2):  # h1 first: longest chain (DMA shift)
                        drain_unit(pqc, h)
                    for ns in range(6):
                        ent = lambda q=pqc, n=ns, **kw: proj_unit(q, n, **kw)
                        if pqc == NQC - 2 and ns >= 6 - PROJ_TAIL_RESERVE:
                            reserve.append(ent)
                        else:
                            queue.append((NR - 1, ent))
            # even pacing: drain the queue by the end; deadlines force early
            npop = max(0, (len(queue) + (NR - 1 - ri)) // max(1, NR - ri))
            while queue and (queue[0][0] <= ri or npop > 0):
                _, thunk = queue.pop(0)
                thunk()
                npop -= 1
            pends.append(cur)
        for pend in pends:
            was_last = emit_av_round(*pend)
            if was_last and pend[0] < NQC - 1:
                for h in (1, 0, 2):
                    drain_unit(pend[0], h)
                for ns in range(6):
                    queue.append((NR - 1, lambda q=pend[0], n=ns, **kw: proj_unit(q, n, **kw)))
        # ---- tail: l-row copies (ACT) first, reserved proj units keep the
        # PE busy under the reciprocal latency, dummy matmuls bridge the
        # final yT wait so proj(3) runs at max clock ----
        for h in (1, 0, 2):
            drain_lcopy(h)
        for h in (1, 0, 2):
            for _ in range(2):
                if reserve:
                    reserve.pop(0)(tail=True)
            drain_unit(NQC - 1, h)
        for _, thunk in queue:
            thunk()
        while reserve:
            reserve.pop(0)(tail=True)
        for ns in range(6):
            proj_unit(NQC - 1, ns, tail=True)

    nc.compile()
    return nc


_NC_CACHE = None


def _get_nc():
    global _NC_CACHE
    if _NC_CACHE is None:
        _NC_CACHE = build_nc()
    return _NC_CACHE


def _host_inputs(x, w_attn, b_attn, w_proj):
    """Per-core input dicts. Core c = batch (c//4), head-group (c%4)."""
    import ml_dtypes

    npm = ml_dtypes.bfloat16
    x = np.ascontiguousarray(np.asarray(x, dtype=np.float32))
    w_attn = np.asarray(w_attn, dtype=np.float32)
    b_attn = np.asarray(b_attn, dtype=np.float32)
    w_proj = np.asarray(w_proj, dtype=np.float32)

    # causal keep-mask tile [128, 128]: 1 where k <= q, else 0 (applied
    # multiplicatively to exp(S) on the diagonal strip)
    pp, ff = np.meshgrid(np.arange(P), np.arange(P), indexing="ij")
    mask = np.where(pp > ff, 0.0, 1.0)

    in_maps = []
    for core in range(N_CORES):
        b, hg = divmod(core, 4)
        hs = 3 * hg  # first head of this core
        # column bases in the 2304-wide qkv dim
        q0, k0, v0 = 64 * hs, CDIM + 64 * hs, 2 * CDIM + 64 * hs
        # M-blocks: b0=[Q0|Q1] b1=[K0|K1] b2=[Q2|K2], then V (192)
        w = np.concatenate(
            [
                w_attn[:, q0 : q0 + 128],
                w_attn[:, k0 : k0 + 128],
                w_attn[:, q0 + 128 : q0 + 192],
                w_attn[:, k0 + 128 : k0 + 192],
                w_attn[:, v0 : v0 + 192],
            ],
            axis=1,
        )
        bqk = np.zeros((3, P), dtype=np.float32)
        bqk[0] = b_attn[q0 : q0 + 128]
        bqk[1] = b_attn[k0 : k0 + 128]
        bqk[2, 0:64] = b_attn[q0 + 128 : q0 + 192]
        bqk[2, 64:128] = b_attn[k0 + 128 : k0 + 192]
        bvb = np.tile(b_attn[v0 : v0 + 192][None, :], (P, 1)).astype(np.float32)
        wp = np.ascontiguousarray(w_proj[64 * hs : 64 * hs + 192, :])
        in_maps.append(
            {
                "x": np.ascontiguousarray(x[b].T.astype(npm)),
                "w": np.ascontiguousarray(w.astype(npm)),
                "bqk": bqk,
                "bvb": bvb,
                "wp": wp.astype(npm),
                "mask": mask.astype(npm),
                "ones_r": np.ones((1, 64), dtype=np.float32),
            }
        )
    return in_maps


def run(x, w_attn, b_attn, w_proj, b_proj, trace=False):
    nc = _get_nc()
    in_maps = _host_inputs(x, w_attn, b_attn, w_proj)
    res = bass_utils.run_bass_kernel_spmd(
        nc, in_maps, core_ids=list(range(N_CORES)), trace=trace
    )
    B = 2
    outT = np.zeros((B, CDIM, T), dtype=np.float64)
    for core in range(N_CORES):
        outT[core // 4] += res.results[core]["out"].astype(np.float64)
    out = outT.transpose(0, 2, 1)
    out = out + np.asarray(b_proj, dtype=np.float64)[None, None, :]
    return np.ascontiguousarray(out.astype(np.float32)), res


def kernel(x, w_attn, b_attn, w_proj, b_proj):
    out, _ = run(x, w_attn, b_attn, w_proj, b_proj, trace=False)
    return out


# revision 41
# speedup vs baseline: 1.2561x; 1.2561x over previous
"""Trainium2 Bass kernel for causal multi-head attention (GPT-style block).

Reference computation (fp32):
    qkv = x @ w_attn + b_attn          # [B,T,3C]
    q,k,v per head (12 heads, d=64)
    att = softmax(causal(q k^T / 8))
    y   = att @ v
    out = y @ w_proj + b_proj

Sharding: 8 cores = 2 batches x 4 head-groups (3 heads each).
Tensor-parallel over heads: each core takes its 3 heads' columns of
w_attn and rows of w_proj, computes a partial out^T [C, T]; the host
sums the 4 head-group partials per batch, transposes, adds b_proj.

Device kernel (per core), all matmuls bf16, PSUM fp32:

  1. Inputs land via few, large DMAs spread over 4 sequencer rings
     (sync/vector/scalar/gpsimd) so dispatch (~0.65us per dma_start)
     isn't serial.  x^T arrives as 6 per-c [128, T] tiles, split
     [0:512] (hot: feeds chunk-0 qkv) + [512:2048].
  2. While inputs stream, the PE runs warm-up matmuls on a zeroed
     scratch tile: the Tensor engine needs ~3us of continuous busy to
     ramp from 1.2GHz to 2.4GHz, so the ramp happens during the DMA
     window instead of during the first real chains.
  3. Q/K^T = w^T x^T in 3 M-blocks: b0=[Q0|Q1] b1=[K0|K1] b2=[Q2|K2]
     ([128,T] SBUF tiles, | = partition 64); bias add on DVE. K2 is
     shifted to partition base 0 via an SBUF-SBUF DMA (matmul requires
     lhsT/rhs partition bases to match).
  4. V computed TOKEN-major directly: V[t,d] = x^T-tile.T @ wv
     -> vk k-tiles [128, 3*65] (64 V cols + ones col per head; the
     ones column turns the AV matmul into a fused softmax-denominator
     sum).
  5. attention rounds r=(qc,j), software-pipelined two rounds deep:
       S^T = K-slice.T @ Q   (PE, causal-sliced N, 4-slot PSUM rotation)
       eS = exp(0.125 S^T)    (ACT - the pacing engine of attention)
       diag strip of eS zeroed by a 0/1 triangle multiply (DVE) AFTER
       the exp, so the exp chain never waits on the mask
       AV of round r-2: y^T[65,q] += [V|1].T @ eS  (PE)
     qkv/V/proj chains are dispatched between rounds from a deadline
     queue, evenly paced, as PE gap fillers (keeps the PE dense).
  6. drain per q-chunk (boundary-latency critical): ONE DVE copy
     moves l + unnormalized y^T out of PSUM so the next chunk's AV can
     reuse the bank immediately; 1/l runs off the critical path
     (mid-kernel: DMA-shift l row to partition 0 + DVE reciprocal +
     GpSimd partition-broadcast), then y^T *= 1/l.  At the tail the
     psy handoff is unnecessary: ACT (idle) copies just the l row to a
     base-64 slot, PE K=1 broadcast, DVE reciprocal, and the normalize
     reads y straight from PSUM.  (Custom-DVE reciprocal and gpsimd
     partition_broadcast are broken on HW at partition base != 0 -
     everything runs at base 0.)
  7. proj emits out^T: pso[128n,512t] = wp-slice.T @ y^T  (PE, N=512),
     engine copy to SBUF, DMA out (rings rotated).  PROJ_TAIL_RESERVE
     proj units are held back to keep the PE busy under the tail
     drain's reciprocal latency.  Host transposes + reduces partials.
"""

import sys

import numpy as np

sys.path.insert(0, "/opt/trn_rl_repo")

from concourse import bacc, bass, mybir  # noqa: E402
from concourse import bass_utils  # noqa: E402
from concourse.tile import TileContext  # noqa: E402

P = 128
T = 2048
CDIM = 768
DHEAD = 64
HPC = 3  # heads per core
N_CORES = 8
FQK = 384  # q+k features per core (3 blocks of 128)
FV = 192  # v features per core
QW = 512  # q-chunk width
NQC = T // QW  # 4
NTT = T // P  # 16 token tiles
NCT = CDIM // P  # 6 contraction tiles
SCALE = 1.0 / np.sqrt(DHEAD)

N_WARM = 13  # PE clock warm-up matmuls; end ~= hot-x arrival so the busy
# streak is unbroken and the qkv chains start at max clock
PROJ_TAIL_RESERVE = 6  # proj units held for the tail drain window

DT = mybir.dt.float32
DTM = mybir.dt.bfloat16


def build_nc():
    from contextlib import ExitStack

    nc = bacc.Bacc("TRN2", target_bir_lowering=False, debug=False)
    x_d = nc.dram_tensor("x", [CDIM, T], DTM, kind="ExternalInput")
    w_d = nc.dram_tensor("w", [CDIM, FQK + FV], DTM, kind="ExternalInput")
    bqk_d = nc.dram_tensor("bqk", [3, P], DT, kind="ExternalInput")
    bvb_d = nc.dram_tensor("bvb", [P, FV], DT, kind="ExternalInput")
    wp_d = nc.dram_tensor("wp", [HPC * DHEAD, CDIM], DTM, kind="ExternalInput")
    m_d = nc.dram_tensor("mask", [P, P], DTM, kind="ExternalInput")
    onr_d = nc.dram_tensor("ones_r", [1, 64], mybir.dt.float32r, kind="ExternalInput")
    o_d = nc.dram_tensor("out", [CDIM, T], DT, kind="ExternalOutput")

    EXP = mybir.ActivationFunctionType.Exp

    with TileContext(nc) as tc, ExitStack() as ctx:
        const = ctx.enter_context(tc.tile_pool(name="const", bufs=1))
        big = ctx.enter_context(tc.tile_pool(name="big", bufs=1))
        work = ctx.enter_context(tc.tile_pool(name="work", bufs=4))
        psyp = ctx.enter_context(
            tc.tile_pool(name="psy", bufs=1, space=bass.MemorySpace.PSUM)
        )
        pssp = ctx.enter_context(
            tc.tile_pool(name="pss", bufs=4, space=bass.MemorySpace.PSUM)
        )
        misc = ctx.enter_context(
            tc.tile_pool(name="misc", bufs=1, space=bass.MemorySpace.PSUM)
        )

        # ---- input DMAs: few + large, spread over 4 rings ----
        # x^T: 6 per-c tiles [128, T]; cols [0:512] first (hot), rest after
        xT = [big.tile([P, T], DTM, tag=f"xT{c}", name=f"xT{c}") for c in range(NCT)]

        def x_ap(c, lo, hi):
            return xT[c][:, lo:hi]

        # warm-up scratch FIRST on the gpsimd ring so the PE can start
        # ramping at t~6.2us with no DMA dependency
        scratch = const.tile([P, QW], DTM, tag="scratch")
        nc.gpsimd.memset(scratch[:], 0.0)

        # Many medium DMAs spread over the 3 DMA-capable rings: each
        # dma_start lands on its own hw queue, so parallelism comes from
        # instruction count, not size.  Hot pieces (w, x t0) dispatch first.
        FW = FQK + FV
        wbig = const.tile([P, NCT * FW], DTM, tag="wbig")
        for c in range(NCT):
            # w c-blocks on scalar (in chain-visit order)
            nc.scalar.dma_start(
                wbig[:, c * FW : (c + 1) * FW], w_d[c * P : (c + 1) * P, :]
            )
        # hot x [0:512]: the minimal set for qkv(0); sync c0-3, gpsimd c4-5
        for c in range(4):
            nc.sync.dma_start(xT[c][:, 0:QW], x_d[c * P : (c + 1) * P, 0:QW])
        for c in range(4, NCT):
            nc.gpsimd.dma_start(xT[c][:, 0:QW], x_d[c * P : (c + 1) * P, 0:QW])
        # small consts on gpsimd (mask gates the first exp round)
        bqk_t = const.tile([P, 3], DT, tag="bqk")
        nc.gpsimd.dma_start(bqk_t[:], bqk_d[:].rearrange("b p -> p b"))
        mask_t = const.tile([P, P], DTM, tag="mask")
        nc.gpsimd.dma_start(mask_t[:], m_d[:])
        bvb_t = const.tile([P, FV], DT, tag="bvb")
        nc.gpsimd.dma_start(bvb_t[:], bvb_d[:])

        # ---- PE warm-up: ramp the Tensor clock during the DMA window ----
        pwarm = pssp.tile([P, QW], DT, tag="pss", name="pwarm")
        for _ in range(N_WARM):
            nc.tensor.matmul(
                pwarm[:], scratch[:, 0:P], scratch[:], start=True, stop=True
            )

        # ---- V k-tiles; ones column per head fused for the l-sum ----
        vk = [
            big.tile([P, HPC * 65], DTM, tag=f"vk{j}", name=f"vk{j}")
            for j in range(NTT)
        ]
        for j in range(NTT):
            ones_view = vk[j][:].rearrange("p (h c) -> p h c", c=65)[:, :, 64:65]
            nc.gpsimd.memset(ones_view, 1.0)

        ones_r = const.tile([65, 64], mybir.dt.float32r, tag="ones_r")
        nc.gpsimd.dma_start(ones_r[64:65, :], onr_d[:])

        # ---- chain emitters (each is a PE filler unit) ----
        blk = [big.tile([P, T], DTM, tag=f"blk{i}", name=f"blk{i}") for i in range(3)]
        # K2 copy at partition base 0 (matmul needs lhsT/rhs bases equal;
        # Q2 lives at base 0 in blk2, K2 at base 64 -> shift via DMA)
        kT2 = big.tile([64, T], DTM, tag="kT2", name="kT2")

        qkv_state = {}

        def qkv_unit(t, bi, half):
            tcols = slice(t * QW, (t + 1) * QW)
            if half == 0:
                qkv_state[(t, bi)] = misc.tile([P, QW], DT, tag="misc", name="psq")
            psq = qkv_state[(t, bi)]
            for c in (half * 2, half * 2 + 1) if half < 2 else (4, 5):
                nc.tensor.matmul(
                    psq[:],
                    wbig[:, c * FW + bi * P : c * FW + (bi + 1) * P],
                    x_ap(c, t * QW, (t + 1) * QW),
                    start=(c == 0),
                    stop=(c == NCT - 1),
                )
            if half == 2:
                nc.vector.tensor_scalar_add(
                    blk[bi][:, tcols], psq[:], bqk_t[:, bi : bi + 1]
                )
                if bi == 2:
                    nc.sync.dma_start(kT2[:, tcols], blk[2][64:P, tcols])
            return 540

        v_state = {}

        def v_unit(j, half):
            if half == 0:
                v_state[j] = misc.tile([P, FV], DT, tag="misc", name="pst")
            pst = v_state[j]
            for c in (half * 2, half * 2 + 1) if half < 2 else (4, 5):
                nc.tensor.matmul(
                    pst[:],
                    x_ap(c, j * P, (j + 1) * P),
                    wbig[:, c * FW + FQK : c * FW + FQK + FV],
                    start=(c == 0),
                    stop=(c == NCT - 1),
                )
            if half == 2:
                data_view = vk[j][:].rearrange("p (h c) -> p h c", c=65)[:, :, 0:64]
                nc.vector.tensor_add(
                    data_view,
                    pst[:].rearrange("p (h c) -> p h c", c=64),
                    bvb_t[:].rearrange("p (h c) -> p h c", c=64),
                )
            return 270

        # ---- attention machinery ----
        qsrc = [(0, 0), (0, 64), (2, 0)]
        ktile = [blk[1], blk[1], kT2]
        krow = [0, 64, 0]
        yT0 = big.tile([P, T], DTM, tag="yT0")  # rows: h0 | h1
        yT1 = big.tile([64, T], DTM, tag="yT1")  # h2
        psy = [None, None, None]

        def emit_s_round(qc, j):
            m = j - 4 * qc
            cs = m * P if m >= 1 else 0
            ssl = slice(cs, QW)
            qsl = slice(qc * QW + cs, (qc + 1) * QW)
            es3 = []
            for h in range(HPC):
                qb, qr = qsrc[h]
                kr = krow[h]
                pss = pssp.tile([P, QW], DT, tag="pss", name="pss")
                nc.tensor.matmul(
                    pss[:, ssl],
                    ktile[h][kr : kr + 64, j * P : (j + 1) * P],
                    blk[qb][qr : qr + 64, qsl],
                    start=True,
                    stop=True,
                )
                es = work.tile([P, QW], DTM, tag="es", bufs=15, name="es")
                nc.scalar.activation(es[:, ssl], pss[:, ssl], EXP, scale=float(SCALE))
                if m >= 0:
                    # causal mask applied AFTER exp (multiply by 0/1
                    # triangle) so the exp never waits on it
                    msl = slice(m * P, (m + 1) * P)
                    nc.vector.tensor_mul(es[:, msl], es[:, msl], mask_t[:])
                es3.append(es)
            return (qc, j, es3, ssl)

        def emit_av_round(qc, j, es3, ssl):
            first = j == 0
            last = j == 4 * (qc + 1) - 1
            if first:
                for h in range(HPC):
                    psy[h] = psyp.tile([65, QW], DT, tag=f"psy{h}", name=f"psy{h}")
            for h in range(HPC):
                nc.tensor.matmul(
                    psy[h][:, ssl],
                    vk[j][:, 65 * h : 65 * h + 65],
                    es3[h][:, ssl],
                    start=first,
                    stop=last,
                )
            return last

        def drain_unit(qc, h):
            # Free psy FAST: one DVE copy moves l + unnormalized y out of
            # PSUM so the next q-chunk's AV can reuse the bank immediately.
            # The 1/l chain (reciprocal at base 0 + broadcast) runs off the
            # critical path; only proj(qc) waits on it.
            qcols = slice(qc * QW, (qc + 1) * QW)
            if h == 0:
                ydst = yT0[0:64, qcols]
            elif h == 2:
                ydst = yT1[0:64, qcols]
            else:
                tmp = work.tile([64, QW], DTM, tag="ytmp", bufs=3)
                ydst = tmp[:]
            rb = work.tile([64, QW], DT, tag="rb", bufs=4)
            if qc < NQC - 1:
                # mid-kernel: keep the broadcast off the PE (the busy
                # engine); DMA-shift l to partition 0, reciprocal there,
                # gpsimd partition-broadcast. Latency is hidden: only
                # proj(qc) waits on rb.
                st = work.tile([65, QW], mybir.dt.float32r, tag="lr", bufs=4)
                nc.vector.tensor_copy(st[:], psy[h][:])
                lr0 = work.tile([1, QW], DT, tag="lr0", bufs=2)
                nc.sync.dma_start(lr0[:], st[64:65, :].bitcast(DT))
                rc = work.tile([1, QW], DT, tag="rc", bufs=2)
                nc.vector.reciprocal_approx_fast(out=rc[:], in_=lr0[:])
                nc.gpsimd.partition_broadcast(rb[:], rc[0:1, :], channels=64)
                nc.vector.tensor_mul(ydst, st[0:64, :].bitcast(DT), rb[:])
            else:
                # tail: latency-critical, psy needs no handoff.  Skip the
                # full copy: ACT (idle now) moves just the l row to a
                # base-64 SBUF slot, PE K=1 broadcast, reciprocal, and
                # normalize reading y straight from PSUM on the DVE.
                lr64 = tail_lr[h]
                rbp = pssp.tile([64, QW], DT, tag="pss", name="rbp")
                nc.tensor.matmul(
                    rbp[:], ones_r[64:65, :], lr64[64:65, :], start=True, stop=True
                )
                nc.vector.reciprocal_approx_fast(out=rb[:], in_=rbp[:])
                nc.vector.tensor_mul(ydst, psy[h][0:64, :], rb[:])
            if h == 1:
                nc.sync.dma_start(yT0[64:P, qcols], tmp[:])
            return 300

        tail_lr = {}

        def drain_lcopy(h):
            # ACT (idle at the tail) moves just the l row to a base-64 slot
            lr64 = work.tile([65, QW], mybir.dt.float32r, tag="lr64", bufs=3)
            nc.scalar.copy(lr64[64:65, :], psy[h][64:65, :])
            tail_lr[h] = lr64

        proj_n = [0]  # running proj-unit counter for ring/engine rotation

        def proj_unit(qc, ns, tail=False):
            tsl = slice(qc * QW, (qc + 1) * QW)
            nsl = slice(ns * P, (ns + 1) * P)
            pool = pssp if tail else misc
            pso = pool.tile([P, QW], DT, tag="pss" if tail else "misc", name="pso")
            nc.tensor.matmul(pso[:], wp0[:, nsl], yT0[:, tsl], start=True, stop=False)
            nc.tensor.matmul(pso[:], wp1[:, nsl], yT1[:, tsl], start=False, stop=True)
            ot = work.tile([P, QW], DT, tag="ot", bufs=4)
            k = proj_n[0]
            proj_n[0] += 1
            if tail:
                # tail: exps done -> ACT is idle; copies mostly on ACT (the
                # DVE carries the drain recip/normalize chain), DMAs
                # alternate sync/scalar
                eng = (nc.scalar.copy, nc.scalar.copy, nc.vector.tensor_copy)[k % 3]
                eng(ot[:], pso[:])
                ring = (nc.sync, nc.scalar)[k % 2]
            else:
                # mid-kernel: ACT is the pacer - keep copies off it (DVE;
                # Pool cannot read PSUM)
                nc.vector.tensor_copy(ot[:], pso[:])
                ring = (nc.sync, nc.gpsimd)[k % 2]
            ring.dma_start(o_d[nsl, tsl], ot[:])
            return 520

        # ---- prologue: only what round (0,0) needs ----
        for bi in range(3):
            for half in range(3):
                qkv_unit(0, bi, half)

        # mid x [512:1024] (qkv(1), rounds 2-3) then cold [1024:2048]
        # (2KB descriptors), spread sync/scalar/gpsimd behind the hot set
        H2 = 2 * QW
        rings = [nc.sync, nc.sync, nc.scalar, nc.scalar, nc.gpsimd, nc.gpsimd]
        for c in range(NCT):
            rings[c].dma_start(
                xT[c][:, QW:H2], x_d[c * P : (c + 1) * P, QW:H2]
            )
        for c in range(NCT):
            rings[c].dma_start(
                xT[c][:, H2:T], x_d[c * P : (c + 1) * P, H2:T]
            )
        wp0 = const.tile([P, CDIM], DTM, tag="wp0")
        nc.gpsimd.dma_start(wp0[:], wp_d[0:P, :])
        wp1 = const.tile([64, CDIM], DTM, tag="wp1")
        nc.gpsimd.dma_start(wp1[:], wp_d[P : P + 64, :])

        def qkv_chain(t, bi):
            for half in range(3):
                qkv_unit(t, bi, half)
            return 1620

        def v_chain(j):
            for half in range(3):
                v_unit(j, half)
            return 810

        # ---- filler queue with deadlines (baseline scheme) ----
        # each entry: (deadline_round_index, thunk). Chains pop atomically;
        # pacing is even distribution (PE total work > ACT total, so the
        # queue must drain by the last round, not just plug ACT gaps).
        # The last drained chunk holds PROJ_TAIL_RESERVE proj units back
        # for the tail, to keep the PE busy under the drain's recip latency.
        rounds = [(qc, j) for qc in range(NQC) for j in range(4 * (qc + 1))]
        ridx = {r: i for i, r in enumerate(rounds)}
        NR = len(rounds)
        queue = []
        for j in range(4):
            queue.append((ridx[(0, j)], lambda j=j: v_chain(j)))
        for t in range(1, 4):
            for bi in range(3):
                queue.append((ridx[(t, 0)] - 2, lambda t=t, bi=bi: qkv_chain(t, bi)))
        for j in range(4, NTT):
            queue.append((ridx[(j // 4, j)], lambda j=j: v_chain(j)))
        queue.sort(key=lambda e: e[0])
        reserve = []

        # ---- main loop: S(r) + AV(r-2) + evenly-paced fillers ----
        pends = []
        for ri, (qc, j) in enumerate(rounds):
            cur = emit_s_round(qc, j)
            # at a qc boundary flush BOTH pending AV rounds so the drain
            # (and the psy bank handoff) starts a full round earlier
            flush = 2 if (j == 0 and pends) else 1
            for _ in range(flush):
                if len(pends) < (3 - flush):
                    break
                if not pends:
                    break
                pend = pends.pop(0)
                was_last = emit_av_round(*pend)
                if was_last:
                    pqc = pend[0]
                    for h in (1, 0, 2):  # h1 first: longest chain (DMA shift)
                        drain_unit(pqc, h)
                    for ns in range(6):
                        ent = lambda q=pqc, n=ns, **kw: proj_unit(q, n, **kw)
                        if pqc == NQC - 2 and ns >= 6 - PROJ_TAIL_RESERVE:
                            reserve.append(ent)
                        else:
                            queue.append((NR - 1, ent))
            # even pacing: drain the queue by the end; deadlines force early
            npop = max(0, (len(queue) + (NR - 1 - ri)) // max(1, NR - ri))
            while queue and (queue[0][0] <= ri or npop > 0):
                _, thunk = queue.pop(0)
                thunk()
                npop -= 1
            pends.append(cur)
        for pend in pends:
            was_last = emit_av_round(*pend)
            if was_last and pend[0] < NQC - 1:
                for h in (1, 0, 2):
                    drain_unit(pend[0], h)
                for ns in range(6):
                    queue.append((NR - 1, lambda q=pend[0], n=ns, **kw: proj_unit(q, n, **kw)))
        # ---- tail: l-row copies (ACT) first, reserved proj units keep the
        # PE busy under the reciprocal latency, dummy matmuls bridge the
        # final yT wait so proj(3) runs at max clock ----
        for h in (1, 0, 2):
            drain_lcopy(h)
        for h in (1, 0, 2):
            for _ in range(2):
                if reserve:
                    reserve.pop(0)(tail=True)
            drain_unit(NQC - 1, h)
        for _, thunk in queue:
            thunk()
        while reserve:
            reserve.pop(0)(tail=True)
        for ns in range(6):
            proj_unit(NQC - 1, ns, tail=True)

    nc.compile()
    return nc


_NC_CACHE = None


def _get_nc():
    global _NC_CACHE
    if _NC_CACHE is None:
        _NC_CACHE = build_nc()
    return _NC_CACHE


def _host_inputs(x, w_attn, b_attn, w_proj):
    """Per-core input dicts. Core c = batch (c//4), head-group (c%4)."""
    import ml_dtypes

    npm = ml_dtypes.bfloat16
    x = np.ascontiguousarray(np.asarray(x, dtype=np.float32))
    w_attn = np.asarray(w_attn, dtype=np.float32)
    b_attn = np.asarray(b_attn, dtype=np.float32)
    w_proj = np.asarray(w_proj, dtype=np.float32)

    # causal keep-mask tile [128, 128]: 1 where k <= q, else 0 (applied
    # multiplicatively to exp(S) on the diagonal strip)
    pp, ff = np.meshgrid(np.arange(P), np.arange(P), indexing="ij")
    mask = np.where(pp > ff, 0.0, 1.0)

    in_maps = []
    for core in range(N_CORES):
        b, hg = divmod(core, 4)
        hs = 3 * hg  # first head of this core
        # column bases in the 2304-wide qkv dim
        q0, k0, v0 = 64 * hs, CDIM + 64 * hs, 2 * CDIM + 64 * hs
        # M-blocks: b0=[Q0|Q1] b1=[K0|K1] b2=[Q2|K2], then V (192)
        w = np.concatenate(
            [
                w_attn[:, q0 : q0 + 128],
                w_attn[:, k0 : k0 + 128],
                w_attn[:, q0 + 128 : q0 + 192],
                w_attn[:, k0 + 128 : k0 + 192],
                w_attn[:, v0 : v0 + 192],
            ],
            axis=1,
        )
        bqk = np.zeros((3, P), dtype=np.float32)
        bqk[0] = b_attn[q0 : q0 + 128]
        bqk[1] = b_attn[k0 : k0 + 128]
        bqk[2, 0:64] = b_attn[q0 + 128 : q0 + 192]
        bqk[2, 64:128] = b_attn[k0 + 128 : k0 + 192]
        bvb = np.tile(b_attn[v0 : v0 + 192][None, :], (P, 1)).astype(np.float32)
        wp = np.ascontiguousarray(w_proj[64 * hs : 64 * hs + 192, :])
        in_maps.append(
            {
                "x": np.ascontiguousarray(x[b].T.astype(npm)),
                "w": np.ascontiguousarray(w.astype(npm)),
                "bqk": bqk,
                "bvb": bvb,
                "wp": wp.astype(npm),
                "mask": mask.astype(npm),
                "ones_r": np.ones((1, 64), dtype=np.float32),
            }
        )
    return in_maps


def run(x, w_attn, b_attn, w_proj, b_proj, trace=False):
    nc = _get_nc()
    in_maps = _host_inputs(x, w_attn, b_attn, w_proj)
    res = bass_utils.run_bass_kernel_spmd(
        nc, in_maps, core_ids=list(range(N_CORES)), trace=trace
    )
    B = 2
    outT = np.zeros((B, CDIM, T), dtype=np.float64)
    for core in range(N_CORES):
        outT[core // 4] += res.results[core]["out"].astype(np.float64)
    out = outT.transpose(0, 2, 1)
    out = out + np.asarray(b_proj, dtype=np.float64)[None, None, :]
    return np.ascontiguousarray(out.astype(np.float32)), res


def kernel(x, w_attn, b_attn, w_proj, b_proj):
    out, _ = run(x, w_attn, b_attn, w_proj, b_proj, trace=False)
    return out
